# revision 14
# baseline (speedup 1.0000x reference)
"""AWQ int4 matmul kernel for Trainium2 (8 NeuronCores, tensor-parallel on out dim).

Computes: out[b,s,o] = sum_k (input[b,s,k]/eq_scales[k]) * ((int_weight-zeros)*scales)[o,k] + bias[o]

Strategy per core (out dim sharded 11008 -> 8 x 1376), v3:
  - Host ships: x as bf16 [2048,4096] (replicated), int4 weights as uint8
    [1376,4096], scales/zeros f32 [1376,32], bias pre-transposed [128,11]
    (bt[p,c]=bias[c*128+p]), eq pre-transposed [128,32]
    (eqt[p,kc]=eq[kc*128+p], the post-xbar K layout).
  - Output is computed TRANSPOSED: out^T [1376, 2048] bf16; host transposes
    and upcasts. This puts out-features on PSUM partitions so the per-o bias
    is a per-partition tensor_scalar, and lets one stationary weight tile
    serve 4 consecutive matmuls (moving = 512-token x slices), cutting PE
    sequencer/LDWEIGHTS overhead (HW-measured 260.6 -> 247.9 ns/MM).
  - x: 16 DMA-xbar transposes straight from HBM into one resident K-major
    tile xres[k%128, kc, t] (128 KB/partition).
  - W per 128-row chunk: uint8 load -> (w - z) exact in bf16 -> * s ->
    DMA-xbar transpose -> wT[k%128, kc, o] -> * (1/eq) in place (eq is
    partition+kc indexed post-transpose; group g == kc since GROUP=128).
  - Warm-up: first 4 chunks run token-slice 0 only (needs just 1/4 of x),
    then their remaining slices, then full 4-slice units; weight prep for
    chunk i+4 overlaps unit i's matmuls.
"""

import sys

sys.path.insert(0, "/opt/trn_rl_repo")

from contextlib import ExitStack

import numpy as np
import ml_dtypes

import concourse.bass as bass
import concourse.mybir as mybir
import concourse.tile as tile
from concourse import bacc
from concourse.bass_utils import run_bass_kernel_spmd

dt = mybir.dt

# Problem shapes (hardcoded per contract).
OUT, IN, GROUP = 11008, 4096, 128
NG = IN // GROUP  # 32
B, S = 2, 1024
T = B * S  # 2048 tokens
N_CORES = 8
O_PC = OUT // N_CORES  # 1376 out features per core
TS = 512  # tokens per moving slice (one PSUM bank)


def build_body(ctx, tc, cfg):
    """Emit the per-core kernel body. cfg: dict with t, in_, o_pc, iters."""
    nc = tc.nc
    P = 128
    T_, IN_, O_ = cfg["t"], cfg["in_"], cfg["o_pc"]
    NG_ = IN_ // GROUP
    n_tc = T_ // P  # x-transpose chunks (16)
    n_ts = T_ // TS  # moving slices (4)

    # out-feature chunks of 128 (tail 96)
    och = []
    o0 = 0
    while o0 < O_:
        ow = min(P, O_ - o0)
        och.append((o0, ow))
        o0 += ow
    n_oc = len(och)

    x_d = nc.dram_tensor("x", [T_, IN_], dt.bfloat16, kind="ExternalInput").ap()
    w_d = nc.dram_tensor("w", [O_, IN_], dt.uint8, kind="ExternalInput").ap()
    s_d = nc.dram_tensor("s", [O_, NG_], dt.float32, kind="ExternalInput").ap()
    z_d = nc.dram_tensor("z", [O_, NG_], dt.float32, kind="ExternalInput").ap()
    bt_d = nc.dram_tensor("bt", [P, n_oc], dt.float32, kind="ExternalInput").ap()
    eqt_d = nc.dram_tensor("eqt", [P, NG_], dt.float32, kind="ExternalInput").ap()
    out_d = nc.dram_tensor("out", [O_, T_], dt.bfloat16, kind="ExternalOutput").ap()

    consts = ctx.enter_context(tc.tile_pool(name="consts", bufs=1))

    def grp_bc(ap_obj, inner=P):
        # [p, NG_] view -> [p, NG_, inner] with stride-0 inner broadcast
        return bass.AP(
            tensor=ap_obj.tensor,
            offset=ap_obj.offset,
            ap=list(ap_obj.ap) + [[0, inner]],
        )

    def as3d(ap_obj, pw):
        # [pw, IN_] tile view -> [pw, NG_, P]
        a = ap_obj.ap
        return bass.AP(
            tensor=ap_obj.tensor,
            offset=ap_obj.offset,
            ap=[[a[0][0], pw], [P, NG_], [1, P]],
        )

    def emit_iter():
        # ---- constants ----
        eqi = consts.tile([P, NG_], dt.float32, tag="eqi")
        nc.gpsimd.dma_start(eqi[:], eqt_d)
        nc.vector.reciprocal(eqi[:], eqi[:])
        b_sb = consts.tile([P, n_oc], dt.float32, tag="b_sb")
        nc.gpsimd.dma_start(b_sb[:], bt_d)

        with ExitStack() as wctx:
            spool = wctx.enter_context(tc.tile_pool(name="sprep", bufs=1))
            xrpool = wctx.enter_context(tc.tile_pool(name="xres", bufs=1))
            wload = wctx.enter_context(tc.tile_pool(name="wload", bufs=2))
            wbf = wctx.enter_context(tc.tile_pool(name="wbf", bufs=1))
            wTp = wctx.enter_context(tc.tile_pool(name="wT", bufs=4))
            wTtp = wctx.enter_context(tc.tile_pool(name="wTt", bufs=1))
            pspool = wctx.enter_context(tc.tile_pool(name="ps", bufs=8, space="PSUM"))
            opool = wctx.enter_context(tc.tile_pool(name="osb", bufs=3))

            # ---- scales/zeros prefetch: [p, oc, g] layout ----
            n_full = sum(1 for (_, ow) in och if ow == P)
            s_all = spool.tile([P, n_oc, NG_], dt.float32, tag="s_all")
            z_all = spool.tile([P, n_oc, NG_], dt.float32, tag="z_all")
            for d_, t_ in ((s_d, s_all), (z_d, z_all)):
                nc.gpsimd.dma_start(
                    t_[:, :n_full, :],
                    bass.AP(
                        tensor=d_.tensor,
                        offset=d_.offset,
                        ap=[[NG_, P], [P * NG_, n_full], [1, NG_]],
                    ),
                )
                if n_oc > n_full:
                    tw = och[-1][1]
                    nc.gpsimd.dma_start(
                        t_[:tw, n_full, :], d_[och[-1][0] : och[-1][0] + tw, :]
                    )

            # resident K-major activations, blocked [p, tcn, kc, t%128] so each
            # x-transpose writes a CONTIGUOUS region (the xbar writes garbage
            # into strided destinations — HW-verified)
            xres = xrpool.tile([P, n_tc, NG_, P], dt.bfloat16, tag="xres")

            def x_load(tcn):
                # ALL transposes on one HWDGE ring: concurrent xbar use from
                # both rings corrupts data (HW-verified)
                nc.sync.dma_start(
                    xres[:, tcn, :, :],
                    x_d[tcn * P : (tcn + 1) * P, :],
                    transpose=True,
                )

            TPB = TS // P  # token-blocks per moving slice (4)

            def xmov(kc, ts):
                # moving operand [128, TPB, 128] = 512 tokens of k-slice kc
                return xres[:, ts * TPB : (ts + 1) * TPB, kc, :]

            def w_prep(oc):
                o0_, ow = och[oc]
                wu = wload.tile([P, IN_], dt.uint8, tag="wu", name="wu")
                nc.gpsimd.dma_start(wu[:ow], w_d[o0_ : o0_ + ow, :])
                wf = wbf.tile([P, IN_], dt.bfloat16, tag="wf", name="wf")
                wu3 = as3d(wu[:ow, :], ow)
                wf3 = as3d(wf[:ow, :], ow)
                # (w - z) is a small integer: exact in bf16; then * s rounds once
                nc.vector.tensor_tensor(
                    wf3, wu3, grp_bc(z_all[:ow, oc, :]), mybir.AluOpType.subtract
                )
                nc.vector.tensor_tensor(
                    wf3, wf3, grp_bc(s_all[:ow, oc, :]), mybir.AluOpType.mult
                )
                # exact-width tile so the transpose destination is contiguous
                if ow == P:
                    wT = wTp.tile([P, NG_, P], dt.bfloat16, tag="wT", name="wT")
                else:
                    wT = wTtp.tile([P, NG_, ow], dt.bfloat16, tag="wTt", name="wTt")
                nc.sync.dma_start(out=wT[:], in_=wf[:ow, :], transpose=True)
                # fold 1/eq into the K-major weights (eq is [p, kc] here)
                nc.vector.tensor_tensor(
                    wT[:], wT[:], grp_bc(eqi[:, :], ow), mybir.AluOpType.mult
                )
                return wT

            def mm_unit(oc, wT, ts_list):
                o0_, ow = och[oc]
                pss = {}
                for ts in ts_list:
                    pss[ts] = pspool.tile([P, TS], dt.float32, tag="ps", name="ps")
                for kc in range(NG_):
                    for ts in ts_list:
                        nc.tensor.matmul(
                            pss[ts][:ow, :],
                            wT[:, kc, :],
                            xmov(kc, ts),
                            start=(kc == 0),
                            stop=(kc == NG_ - 1),
                        )
                for ts in ts_list:
                    ob = opool.tile([P, TS], dt.bfloat16, tag="ob", name="ob")
                    nc.vector.tensor_scalar_add(
                        ob[:ow, :], pss[ts][:ow, :], b_sb[:ow, oc : oc + 1]
                    )
                    nc.gpsimd.dma_start(
                        out_d[o0_ : o0_ + ow, ts * TS : (ts + 1) * TS], ob[:ow, :]
                    )

            # ---- emission schedule ----
            wTs = {}

            def ensure(oc):
                if oc < n_oc and oc not in wTs:
                    wTs[oc] = w_prep(oc)

            n_a = min(4, n_oc)  # chunks that run ts0 first while x loads
            for tcn in range(min(TS // P, n_tc)):  # tokens 0..TS-1
                x_load(tcn)
            ensure(0)
            ensure(1)
            for tcn in range(TS // P, min(10, n_tc)):
                x_load(tcn)
            ensure(2)
            for tcn in range(10, n_tc):
                x_load(tcn)
            # A-units: token-slice 0 only (x beyond slice 0 still loading).
            # NOTE: chunk i+4 reuses wT slot i (bufs=4), so its prep may only
            # be emitted after the LAST reader of wT[i] (B-unit i) — else the
            # WAR edge points backwards and Tile deadlocks.
            for i in range(n_a):
                ensure(i)
                mm_unit(i, wTs[i], [0])
            # B-units: remaining slices for the A chunks
            rest = list(range(1, n_ts))
            if rest:
                for i in range(n_a):
                    mm_unit(i, wTs[i], rest)
                    ensure(n_a + i)
            # F-units: full sweeps
            for oc in range(n_a, n_oc):
                ensure(oc)
                mm_unit(oc, wTs[oc], list(range(n_ts)))
                ensure(oc + 4)

    iters = cfg.get("iters", 1)
    if iters == 1:
        emit_iter()
    else:
        # big body (>256 instructions/engine): arm branch prefetch so the
        # back-edge I$-hits instead of stalling ~4us per engine per iteration
        hints = (
            mybir.EngineType.PE,
            mybir.EngineType.DVE,
            mybir.EngineType.SP,
            mybir.EngineType.Activation,
            mybir.EngineType.Pool,
        )
        with tc.For_i(0, iters, 1, hint_engines=hints):
            emit_iter()


def build(t=T, in_=IN, o_pc=O_PC, iters=1, compile_=True, debug=False):
    cfg = dict(t=t, in_=in_, o_pc=o_pc, iters=iters, debug=debug)
    nc = bacc.Bacc("TRN2", target_bir_lowering=False, debug=False)
    with tile.TileContext(nc) as tc, ExitStack() as ctx:
        build_body(ctx, tc, cfg)
    if compile_:
        nc.compile()
    return nc


def make_in_maps(input, int_weight, scales, zeros, eq_scales, bias, n_cores=N_CORES):
    """Shard full inputs -> per-core input maps (host-side slicing/packing)."""
    t = input.shape[0] * input.shape[1]
    in_ = input.shape[2]
    o_pc = int_weight.shape[0] // n_cores
    ng = in_ // GROUP
    n_oc = (o_pc + P_HOST - 1) // P_HOST
    x2d = np.ascontiguousarray(
        np.asarray(input, np.float32).reshape(t, in_)
    ).astype(ml_dtypes.bfloat16)
    w_u8 = np.asarray(int_weight).reshape(OUT, in_).astype(np.uint8)
    s2 = np.asarray(scales, np.float32).reshape(OUT, ng)
    z2 = np.asarray(zeros).reshape(OUT, ng).astype(np.float32)
    b1 = np.asarray(bias, np.float32)
    eqt = np.ascontiguousarray(
        np.asarray(eq_scales, np.float32).reshape(ng, GROUP).T
    )
    in_maps = []
    for c in range(n_cores):
        sl = slice(c * o_pc, (c + 1) * o_pc)
        bpad = np.zeros(n_oc * P_HOST, np.float32)
        bpad[:o_pc] = b1[sl]
        bt = bpad.reshape(n_oc, P_HOST).T.copy()  # bt[p, oc] = bias[oc*128+p]
        in_maps.append(
            {
                "x": x2d,
                "w": np.ascontiguousarray(w_u8[sl]),
                "s": np.ascontiguousarray(s2[sl]),
                "z": np.ascontiguousarray(z2[sl]),
                "bt": np.ascontiguousarray(bt),
                "eqt": eqt,
            }
        )
    return in_maps


P_HOST = 128

_NC_CACHE = {}


def kernel(input, int_weight, scales, zeros, eq_scales, bias):
    """Full-input entry point: shard, run on 8 cores, gather."""
    key = ("main", 1)
    if key not in _NC_CACHE:
        _NC_CACHE[key] = build()
    nc = _NC_CACHE[key]
    in_maps = make_in_maps(input, int_weight, scales, zeros, eq_scales, bias)
    # First execution after NEFF load runs with cold engine caches; execute
    # twice and return the warm result.
    run_bass_kernel_spmd(nc, in_maps, list(range(N_CORES)))
    res = run_bass_kernel_spmd(nc, in_maps, list(range(N_CORES)))
    outs = [np.asarray(res.results[c]["out"]) for c in range(N_CORES)]
    full = np.concatenate(outs, axis=0).astype(np.float32).T
    return np.ascontiguousarray(full).reshape(B, S, OUT)
